# revision 1
# baseline (speedup 1.0000x reference)
"""DCT-II enhancement kernel for Trainium2 (8 NeuronCores, data parallel).

Computes out[b, n, k] = sum_d x[b, n, d] * C[k, d] where C is the 256x256
orthonormal DCT-II basis — i.e. a [B*N, 256] @ [256, 256]^T GEMM.

Sharding: pure data parallel over the flattened token dim (B*N = 131072),
16384 tokens per core. The DCT basis (transposed, [d, k]) and a 128x128
identity (for PE-transpose) are replicated to every core.

Per-core dataflow, per 512-token super-tile:
  1. DMA x tile [128p(tok), 4t, 256d] from HBM (natural layout, contiguous).
  2. PE-transpose (fp32r) the 8 [128, 128] blocks -> xT in PSUM [d, tok].
  3. Copy PSUM -> SBUF (DVE).
  4. fp32r matmuls: out[tok=128, k=256] += xT_chunk.T @ CT_chunk for the
     two 128-deep d-chunks (moving free dim 256 -> full-rate fp32r).
  5. Copy PSUM -> SBUF (DVE/ACT), DMA out to HBM in natural layout.
"""

from contextlib import ExitStack

import numpy as np

import concourse.bass as bass
import concourse.tile as tile
from concourse import bacc, mybir
from concourse.bass_utils import run_bass_kernel_spmd

P = 128
D = 256
N_CORES = 8
B, N = 32, 4096
TOK_PER_CORE = (B * N) // N_CORES  # 16384

F32 = mybir.dt.float32
F32R = mybir.dt.float32r


def dct_matrix() -> np.ndarray:
    """C[k, d] — DCT-II with ortho normalization, fp64 math cast to fp32."""
    n = D
    k = np.arange(n)[:, None].astype(np.float64)
    m = np.arange(n)[None, :].astype(np.float64)
    Cm = np.cos(np.pi * (2.0 * m + 1.0) * k / (2.0 * n))
    scale = np.full((n, 1), np.sqrt(2.0 / n))
    scale[0, 0] = np.sqrt(1.0 / n)
    return (Cm * scale).astype(np.float32)


def build_program(tok: int = TOK_PER_CORE, super_tok: int = 512,
                  num_devices: int = N_CORES) -> bass.Bass:
    """Emit the per-core Bass/Tile program. All cores run the same NEFF."""
    assert tok % super_tok == 0 and super_tok % P == 0
    nit = tok // super_tok   # super-tile iterations
    tb = super_tok // P      # token-blocks per super-tile
    dc = D // P              # d-chunks (contraction over 2x128)

    nc = bacc.Bacc(
        "TRN2", target_bir_lowering=False, debug=False, num_devices=num_devices
    )
    x_d = nc.dram_tensor("x", [tok, D], F32, kind="ExternalInput").ap()
    ct_d = nc.dram_tensor("ct", [D, D], F32, kind="ExternalInput").ap()
    id_d = nc.dram_tensor("ident", [P, P], F32, kind="ExternalInput").ap()
    out_d = nc.dram_tensor("out", [tok, D], F32, kind="ExternalOutput").ap()

    with ExitStack() as ctx:
        tc = ctx.enter_context(tile.TileContext(nc))
        consts = ctx.enter_context(tc.tile_pool(name="consts", bufs=1))
        xin_pool = ctx.enter_context(tc.tile_pool(name="xin", bufs=3))
        xt_sb_pool = ctx.enter_context(tc.tile_pool(name="xt_sb", bufs=3))
        out_sb_pool = ctx.enter_context(tc.tile_pool(name="out_sb", bufs=3))
        xt_ps_pool = ctx.enter_context(
            tc.tile_pool(name="xt_ps", bufs=4, space="PSUM")
        )
        out_ps_pool = ctx.enter_context(
            tc.tile_pool(name="out_ps", bufs=4, space="PSUM")
        )

        # Replicated constants: CT as [p, c, k] (d = c*128 + p), identity.
        ct_sb = consts.tile([P, dc, D], F32R)
        nc.sync.dma_start(
            ct_sb[:], ct_d.rearrange("(c p) k -> p c k", p=P).bitcast(F32R)
        )
        ident = consts.tile([P, P], F32R)
        nc.sync.dma_start(ident[:], id_d.bitcast(F32R))

        # token = (i*tb + t)*128 + p
        x_t = x_d.rearrange("(i t p) d -> i p t d", p=P, t=tb)
        o_t = out_d.rearrange("(i t p) k -> i p t k", p=P, t=tb)

        for i in range(nit):
            xin = xin_pool.tile([P, tb, D], F32R)
            nc.sync.dma_start(xin[:], x_t[i].bitcast(F32R))

            # xT[d_chunk c][p=d, t*128+m] = x[tok, d]
            xt_sb = xt_sb_pool.tile([P, dc, super_tok], F32R)
            for c in range(dc):
                xt_ps = xt_ps_pool.tile([P, super_tok], F32R)
                for t in range(tb):
                    nc.tensor.transpose(
                        xt_ps[:, t * P:(t + 1) * P],
                        xin[:, t, c * P:(c + 1) * P],
                        ident[:],
                    )
                nc.vector.tensor_copy(xt_sb[:, c, :], xt_ps[:])

            out_sb = out_sb_pool.tile([P, tb, D], F32)
            for t in range(tb):
                out_ps = out_ps_pool.tile([P, D], F32)
                for c in range(dc):
                    nc.tensor.matmul(
                        out_ps[:],
                        xt_sb[:, c, t * P:(t + 1) * P],
                        ct_sb[:, c, :],
                        start=(c == 0),
                        stop=(c == dc - 1),
                    )
                if t % 2 == 0:
                    nc.vector.tensor_copy(out_sb[:, t, :], out_ps[:])
                else:
                    nc.scalar.copy(out_sb[:, t, :], out_ps[:])
            nc.sync.dma_start(o_t[i], out_sb[:])

    nc.compile()
    return nc


_PROGRAM_CACHE: dict = {}


def _get_program() -> bass.Bass:
    if "nc" not in _PROGRAM_CACHE:
        _PROGRAM_CACHE["nc"] = build_program()
    return _PROGRAM_CACHE["nc"]


def make_in_maps(x_flat: np.ndarray) -> list[dict]:
    ct = np.ascontiguousarray(dct_matrix().T)  # [d, k]
    ident = np.eye(P, dtype=np.float32)
    shards = x_flat.reshape(N_CORES, TOK_PER_CORE, D)
    return [
        {"x": np.ascontiguousarray(shards[i]), "ct": ct, "ident": ident}
        for i in range(N_CORES)
    ]


def kernel(x: np.ndarray) -> np.ndarray:
    x = np.ascontiguousarray(np.asarray(x, dtype=np.float32))
    b, n, d = x.shape
    assert (b, n, d) == (B, N, D), f"unexpected shape {x.shape}"
    nc = _get_program()
    in_maps = make_in_maps(x.reshape(b * n, d))
    res = run_bass_kernel_spmd(nc, in_maps, core_ids=list(range(N_CORES)))
    out = np.concatenate([r["out"] for r in res.results], axis=0)
    return out.reshape(b, n, d)


# revision 2
# speedup vs baseline: 1.1375x; 1.1375x over previous
"""DCT-II enhancement kernel for Trainium2 (8 NeuronCores, data parallel).

Computes out[b, n, k] = sum_d x[b, n, d] * C[k, d] where C is the 256x256
orthonormal DCT-II basis — i.e. a [B*N, 256] @ [256, 256]^T GEMM.

Sharding: pure data parallel over the flattened token dim (B*N = 131072),
16384 tokens per core. The DCT basis (transposed, [d, k]) and a 128x128
identity (for PE-transpose) are replicated to every core.

Per-core dataflow, per 512-token super-tile:
  1. DMA x tile [128p(tok), 4t, 256d] from HBM (natural layout, contiguous).
  2. PE-transpose (fp32r) the 8 [128, 128] blocks -> xT in PSUM [d, tok].
  3. Copy PSUM -> SBUF (DVE).
  4. fp32r matmuls: out[tok=128, k=256] += xT_chunk.T @ CT_chunk for the
     two 128-deep d-chunks (moving free dim 256 -> full-rate fp32r).
  5. Copy PSUM -> SBUF (DVE/ACT), DMA out to HBM in natural layout.
"""

from contextlib import ExitStack

import numpy as np

import concourse.bass as bass
import concourse.tile as tile
from concourse import bacc, mybir
from concourse.bass_utils import run_bass_kernel_spmd

P = 128
D = 256
N_CORES = 8
B, N = 32, 4096
TOK_PER_CORE = (B * N) // N_CORES  # 16384

F32 = mybir.dt.float32
F32R = mybir.dt.float32r


def dct_matrix() -> np.ndarray:
    """C[k, d] — DCT-II with ortho normalization, fp64 math cast to fp32."""
    n = D
    k = np.arange(n)[:, None].astype(np.float64)
    m = np.arange(n)[None, :].astype(np.float64)
    Cm = np.cos(np.pi * (2.0 * m + 1.0) * k / (2.0 * n))
    scale = np.full((n, 1), np.sqrt(2.0 / n))
    scale[0, 0] = np.sqrt(1.0 / n)
    return (Cm * scale).astype(np.float32)


def build_program(tok: int = TOK_PER_CORE, super_tok: int = 512,
                  num_devices: int = N_CORES) -> bass.Bass:
    """Emit the per-core Bass/Tile program. All cores run the same NEFF.

    Layout: token = i*super_tok + p*tb + s  (tb tokens per partition, so
    each partition's DMA run is tb*D*4 bytes contiguous — 4 KB at tb=4).
    Emission is software-pipelined: iteration i's transposes are emitted
    before iteration i-1's matmuls so the PE never idles during the
    PSUM->SBUF copy round-trips (keeps HAM warm at 2.4 GHz).
    """
    assert tok % super_tok == 0 and super_tok % P == 0
    nit = tok // super_tok   # super-tile iterations
    tb = super_tok // P      # tokens per partition per super-tile
    dc = D // P              # d-chunks (contraction over 2x128)

    nc = bacc.Bacc(
        "TRN2", target_bir_lowering=False, debug=False, num_devices=num_devices
    )
    x_d = nc.dram_tensor("x", [tok, D], F32, kind="ExternalInput").ap()
    ct_d = nc.dram_tensor("ct", [D, D], F32, kind="ExternalInput").ap()
    id_d = nc.dram_tensor("ident", [P, P], F32, kind="ExternalInput").ap()
    out_d = nc.dram_tensor("out", [tok, D], F32, kind="ExternalOutput").ap()

    with ExitStack() as ctx:
        tc = ctx.enter_context(tile.TileContext(nc))
        consts = ctx.enter_context(tc.tile_pool(name="consts", bufs=1))
        xin_pool = ctx.enter_context(tc.tile_pool(name="xin", bufs=4))
        xt_sb_pool = ctx.enter_context(tc.tile_pool(name="xt_sb", bufs=3))
        out_sb_pool = ctx.enter_context(tc.tile_pool(name="out_sb", bufs=3))
        xt_ps_pool = ctx.enter_context(
            tc.tile_pool(name="xt_ps", bufs=4, space="PSUM")
        )
        out_ps_pool = ctx.enter_context(
            tc.tile_pool(name="out_ps", bufs=4, space="PSUM")
        )

        # Replicated constants: CT as [p, c, k] (d = c*128 + p), identity.
        ct_sb = consts.tile([P, dc, D], F32R)
        nc.sync.dma_start(
            ct_sb[:], ct_d.rearrange("(c p) k -> p c k", p=P).bitcast(F32R)
        )
        ident = consts.tile([P, P], F32R)
        nc.sync.dma_start(ident[:], id_d.bitcast(F32R))

        # token = i*super_tok + p*tb + s -> per-partition contiguous tb*D run
        x_t = x_d.rearrange("(i p s) d -> i p s d", p=P, s=tb)
        o_t = out_d.rearrange("(i p s) k -> i p s k", p=P, s=tb)

        def emit_front(i):
            """DMA in + transposes + xT copies for iteration i."""
            xin = xin_pool.tile([P, tb, D], F32R)
            nc.sync.dma_start(xin[:], x_t[i].bitcast(F32R))

            # xT[c][p=d, s*128+m] = x[token(m, s), c*128+p]
            xt_sb = xt_sb_pool.tile([P, dc, super_tok], F32R)
            for c in range(dc):
                xt_ps = xt_ps_pool.tile([P, super_tok], F32R)
                for s in range(tb):
                    nc.tensor.transpose(
                        xt_ps[:, s * P:(s + 1) * P],
                        xin[:, s, c * P:(c + 1) * P],
                        ident[:],
                    )
                nc.vector.tensor_copy(xt_sb[:, c, :], xt_ps[:])
            return xt_sb

        def emit_back(i, xt_sb):
            """Matmuls + out copies + DMA out for iteration i."""
            out_sb = out_sb_pool.tile([P, tb, D], F32)
            for s in range(tb):
                out_ps = out_ps_pool.tile([P, D], F32)
                for c in range(dc):
                    nc.tensor.matmul(
                        out_ps[:],
                        xt_sb[:, c, s * P:(s + 1) * P],
                        ct_sb[:, c, :],
                        start=(c == 0),
                        stop=(c == dc - 1),
                    )
                if s % 2 == 0:
                    nc.vector.tensor_copy(out_sb[:, s, :], out_ps[:])
                else:
                    nc.scalar.copy(out_sb[:, s, :], out_ps[:])
            nc.scalar.dma_start(o_t[i], out_sb[:])

        prev = None
        for i in range(nit):
            xt_sb = emit_front(i)
            if prev is not None:
                emit_back(prev[0], prev[1])
            prev = (i, xt_sb)
        emit_back(prev[0], prev[1])

    nc.compile()
    return nc


_PROGRAM_CACHE: dict = {}


def _get_program() -> bass.Bass:
    if "nc" not in _PROGRAM_CACHE:
        _PROGRAM_CACHE["nc"] = build_program()
    return _PROGRAM_CACHE["nc"]


def make_in_maps(x_flat: np.ndarray) -> list[dict]:
    ct = np.ascontiguousarray(dct_matrix().T)  # [d, k]
    ident = np.eye(P, dtype=np.float32)
    shards = x_flat.reshape(N_CORES, TOK_PER_CORE, D)
    return [
        {"x": np.ascontiguousarray(shards[i]), "ct": ct, "ident": ident}
        for i in range(N_CORES)
    ]


def kernel(x: np.ndarray) -> np.ndarray:
    x = np.ascontiguousarray(np.asarray(x, dtype=np.float32))
    b, n, d = x.shape
    assert (b, n, d) == (B, N, D), f"unexpected shape {x.shape}"
    nc = _get_program()
    in_maps = make_in_maps(x.reshape(b * n, d))
    res = run_bass_kernel_spmd(nc, in_maps, core_ids=list(range(N_CORES)))
    out = np.concatenate([r["out"] for r in res.results], axis=0)
    return out.reshape(b, n, d)
